# revision 7
# baseline (speedup 1.0000x reference)
"""Trainium2 Bass kernel for Conv2D (1x1) multi-head attention block.

Reference computation (per batch image of [64, 64, 512] = [N=4096, C=512]):
    x  = GroupNorm(inputs, G=32, eps=1e-6) * gamma + beta
    q, k, v = x @ wq + bq, x @ wk + bk, x @ wv + bv      (1x1 convs)
    scores  = (q / sqrt(C)) @ k^T                         [N, N]
    out     = softmax(scores) @ v @ wo + bo + inputs

Sharding: 8 cores = 2 batches x 4 query-quarters. Each core holds the full
image of its batch (needed for GroupNorm stats and full-attention K/V) and
computes the output rows of its query quarter only.

Key implementation choices:
  - GroupNorm is folded into the projection weights: with per-channel
    a[c] = gamma*rstd, b[c] = beta - mean*gamma*rstd, we have
    K^T = (diag(a) wk)^T x^T + (wk^T b + bk) 1^T, so the normalized
    activations are never materialized.
  - All big matmuls run in float32r (TF32-like, 1 cycle/row vs 4 for fp32).
  - Attention uses the transposed-scores layout: scores^T[k, q] tiles come
    straight from matmul(lhsT=K^T tile, rhs=Q^T), exp runs on the scalar
    engine PSUM->SBUF, and probs^T feeds matmul(lhsT=V tile, rhs=probs^T)
    accumulating attn^T[c, q] in PSUM over all 32 key tiles.  Softmax row
    sums come from an extra ones-column matmul; no max-subtraction is needed
    because scores are O(1) by construction (q is pre-scaled by 1/sqrt(C)).
  - x^T is produced by PE transposes of 128x128 blocks, in two streaming
    passes (one for K/Q, one for V) so K^T, V and Q^T can stay resident in
    SBUF within the 24MB budget.
"""

import sys

sys.path.insert(0, "/opt/trn_rl_repo")

from contextlib import ExitStack

import numpy as np

import concourse.bacc as bacc
import concourse.tile as tile
from concourse import mybir
from concourse.bass_utils import run_bass_kernel_spmd

# Problem shape (hardcoded; kernel.py must be self-contained).
B, HH, WW, C = 2, 64, 64, 512
N = HH * WW          # 4096 pixels per batch image
G = 32               # groupnorm groups
GS = C // G          # 16 channels per group
EPS = 1e-6
P = 128              # partitions
CT = C // P          # 4 channel tiles
NT = N // P          # 32 pixel tiles per image
CHUNK = 512          # free-dim chunk for moving operands
NCH = N // CHUNK     # 8 pixel chunks per image
NCORES = 8
QS = N // 4          # 1024 query rows per core
QTILES = QS // P     # 8 query tiles per core
QCH = QS // CHUNK    # 2 query chunks per core
GROUP_COUNT = N * GS  # elements per (batch, group) for the mean/var

F32 = mybir.dt.float32
F32R = mybir.dt.float32r
BF16 = mybir.dt.bfloat16
AF = mybir.ActivationFunctionType

# Whether DMA may write float32r tiles directly (PE truncates on read).
# If the BIR verifier rejects un-rounded producers, set False: inputs are
# then loaded as f32 and every f32r operand is produced by ACT/DVE copies.
DIRECT_F32R_DMA = True

_NC_CACHE = None


def _in_dt():
    return F32R if DIRECT_F32R_DMA else F32


def _build():
    nc = bacc.Bacc(None, target_bir_lowering=False, debug=False)

    xdt = _in_dt()
    x_full = nc.dram_tensor("x_full", [N, C], xdt, kind="ExternalInput")
    x_res = nc.dram_tensor("x_res", [QS, C], F32, kind="ExternalInput")
    x_resr = nc.dram_tensor("x_resr", [QS, C], xdt, kind="ExternalInput")
    gamma_d = nc.dram_tensor("gamma", [C], F32, kind="ExternalInput")
    beta_d = nc.dram_tensor("beta", [C], F32, kind="ExternalInput")
    w_d = {}
    b_d = {}
    for nm in ("wq", "wk", "wv", "wo"):
        w_d[nm] = nc.dram_tensor(nm, [C, C], F32, kind="ExternalInput")
    for nm in ("bq", "bk", "bv", "bo"):
        b_d[nm] = nc.dram_tensor(nm, [C], F32, kind="ExternalInput")
    ident_d = nc.dram_tensor("ident", [P, P], xdt, kind="ExternalInput")
    gind_d = nc.dram_tensor("gind", [P, 8], F32, kind="ExternalInput")
    gindt_d = nc.dram_tensor("gindt", [8, P], F32, kind="ExternalInput")
    out_d = nc.dram_tensor("out", [QS, C], F32, kind="ExternalOutput")

    with tile.TileContext(nc) as tc, ExitStack() as top:
        # ---- persistent pools (live for the whole kernel) ----
        consts = top.enter_context(tc.tile_pool(name="consts", bufs=1))
        pkt = top.enter_context(tc.tile_pool(name="pkt", bufs=1))
        pqt = top.enter_context(tc.tile_pool(name="pqt", bufs=1))
        pv = top.enter_context(tc.tile_pool(name="pv", bufs=1))
        pmisc = top.enter_context(tc.tile_pool(name="pmisc", bufs=1))

        ident = consts.tile([P, P], xdt, name="ident")
        nc.sync.dma_start(out=ident, in_=ident_d[:])
        gind = consts.tile([P, 8], F32, name="gind")
        nc.sync.dma_start(out=gind, in_=gind_d[:])
        gindt = consts.tile([8, P], F32, name="gindt")
        nc.sync.dma_start(out=gindt, in_=gindt_d[:])
        ones_f32 = consts.tile([P, 1], F32, name="ones_f32")
        nc.vector.memset(ones_f32, 1.0)
        ones_r = consts.tile([P, 1], F32R, name="ones_r")
        nc.scalar.copy(ones_r, ones_f32)
        ones_bf = consts.tile([P, 1], BF16, name="ones_bf")
        nc.scalar.copy(ones_bf, ones_f32)
        one11 = ones_f32[0:1, 0:1]

        gamma4 = []
        beta4 = []
        for ct in range(CT):
            gt_ = consts.tile([P, 1], F32, name=f"gamma4_{ct}")
            nc.sync.dma_start(out=gt_, in_=gamma_d[ct * P:(ct + 1) * P])
            gamma4.append(gt_)
            bt_ = consts.tile([P, 1], F32, name=f"beta4_{ct}")
            nc.sync.dma_start(out=bt_, in_=beta_d[ct * P:(ct + 1) * P])
            beta4.append(bt_)

        # K^T, Q^T resident (f32r): [cout-tile][128, n]
        kt = [pkt.tile([P, N], F32R, name=f"kt{i}", tag=f"kt{i}") for i in range(CT)]
        qt = [pqt.tile([P, QS], F32R, name=f"qt{i}", tag=f"qt{i}") for i in range(CT)]
        # V natural resident (f32r): [pixel-tile][128 pixels, C]
        vv = [pv.tile([P, C], BF16, name=f"v{i}", tag=f"v{i}") for i in range(NT)]

        # ================= Phase A: groupnorm statistics =================
        # Per-channel sums of x and x^2 via ones-matmuls over natural tiles.
        with tc.tile_pool(name="pa", bufs=3) as pa, \
             tc.tile_pool(name="psa", bufs=1, space="PSUM") as psa:
            s_ps = psa.tile([1, C], F32, name="s_ps", tag="s_ps")
            sq_ps = psa.tile([1, C], F32, name="sq_ps", tag="sq_ps")
            for it in range(NT):
                xa = pa.tile([P, C], xdt, name="xa", tag="xa")
                nc.sync.dma_start(out=xa, in_=x_full[it * P:(it + 1) * P, :])
                x2 = pa.tile([P, C], F32R, name="x2", tag="x2")
                nc.vector.tensor_mul(x2, xa, xa)
                if DIRECT_F32R_DMA:
                    xs = xa
                else:
                    xs = pa.tile([P, C], F32R, name="xs", tag="xs")
                    nc.scalar.copy(xs, xa)
                nc.tensor.matmul(s_ps, lhsT=ones_r, rhs=xs,
                                 start=(it == 0), stop=(it == NT - 1))
                nc.tensor.matmul(sq_ps, lhsT=ones_r, rhs=x2,
                                 start=(it == 0), stop=(it == NT - 1))

            # ============ Phase B: finalize stats, fold coefficients ======
            s_sb = pmisc.tile([1, C], F32, name="s_sb")
            nc.vector.tensor_copy(s_sb, s_ps)
            sq_sb = pmisc.tile([1, C], F32, name="sq_sb")
            nc.vector.tensor_copy(sq_sb, sq_ps)

        a4 = []   # per-channel scale, partition-major per channel tile
        aq4 = []  # a4 * 1/sqrt(C) for the query projection
        b4 = []   # per-channel shift
        with tc.tile_pool(name="psb", bufs=2, space="PSUM") as psb, \
             tc.tile_pool(name="pb", bufs=2) as pb:
            for ct in range(CT):
                # replicate free-major sums into partition-major [128, 2]
                st4 = psb.tile([P, 2], F32, name="st4", tag="st4")
                nc.tensor.matmul(st4[:, 0:1], lhsT=s_sb[0:1, ct * P:(ct + 1) * P],
                                 rhs=one11, start=True, stop=True)
                nc.tensor.matmul(st4[:, 1:2], lhsT=sq_sb[0:1, ct * P:(ct + 1) * P],
                                 rhs=one11, start=True, stop=True)
                st4_sb = pb.tile([P, 2], F32, name="st4_sb", tag="st4_sb")
                nc.vector.tensor_copy(st4_sb, st4)
                # per-group sums for the 8 groups in this channel tile
                grp_ps = psb.tile([8, 2], F32, name="grp_ps", tag="grp_ps")
                nc.tensor.matmul(grp_ps, lhsT=gind, rhs=st4_sb, start=True, stop=True)
                grp = pb.tile([8, 2], F32, name="grp", tag="grp")
                # E[x], E[x^2]
                nc.vector.tensor_scalar_mul(grp, grp_ps, 1.0 / GROUP_COUNT)
                var = pb.tile([8, 1], F32, name="var", tag="var")
                nc.vector.tensor_mul(var, grp[:, 0:1], grp[:, 0:1])
                nc.vector.tensor_sub(var, grp[:, 1:2], var)
                nc.vector.tensor_scalar_add(var, var, EPS)
                rstd = pb.tile([8, 1], F32, name="rstd", tag="rstd")
                nc.vector.reciprocal(rstd, var)
                nc.scalar.sqrt(rstd, rstd)
                mr = pb.tile([8, 2], F32, name="mr", tag="mr")
                nc.vector.tensor_copy(mr[:, 0:1], grp[:, 0:1])
                nc.vector.tensor_copy(mr[:, 1:2], rstd)
                # broadcast group stats back to channels (partition-major)
                mch_ps = psb.tile([P, 2], F32, name="mch_ps", tag="mch_ps")
                nc.tensor.matmul(mch_ps, lhsT=gindt, rhs=mr, start=True, stop=True)
                mch = pb.tile([P, 2], F32, name="mch", tag="mch")
                nc.vector.tensor_copy(mch, mch_ps)
                a_t = pmisc.tile([P, 1], F32, name=f"a4_{ct}")
                nc.vector.tensor_mul(a_t, gamma4[ct], mch[:, 1:2])
                a4.append(a_t)
                aq_t = pmisc.tile([P, 1], F32, name=f"aq4_{ct}")
                nc.vector.tensor_scalar_mul(aq_t, a_t, 1.0 / float(np.sqrt(C)))
                aq4.append(aq_t)
                b_t = pmisc.tile([P, 1], F32, name=f"b4_{ct}")
                nc.vector.tensor_mul(b_t, mch[:, 0:1], a_t)
                nc.vector.tensor_sub(b_t, beta4[ct], b_t)
                b4.append(b_t)

        # ============ Phase C1+D1: fold wk/wq, transpose x, project K/Q ====
        def fold_weight(nm, scales, qscale, pool, pspool, wpool):
            """Load w, scale rows by a[cin] -> f32r tiles; return folded
            weight tiles and the folded bias [1, C] (partition-major [P,1]x4)."""
            wf = []
            raws = []
            for ct in range(CT):
                raw = wpool.tile([P, C], F32, name=f"{nm}_raw", tag=f"{nm}_raw")
                nc.sync.dma_start(out=raw, in_=w_d[nm][ct * P:(ct + 1) * P, :])
                raws.append(raw)
                wf_t = pool.tile([P, C], F32R, name=f"{nm}_f{ct}", tag=f"{nm}_f{ct}")
                nc.scalar.mul(wf_t, raw, scales[ct])
                wf.append(wf_t)
            # bias' = bias + w^T b  (+ optional 1/sqrt(C) scale for q)
            bias_ps = pspool.tile([1, C], F32, name=f"{nm}_bps", tag="bias_ps")
            for ct in range(CT):
                nc.tensor.matmul(bias_ps, lhsT=b4[ct], rhs=raws[ct],
                                 start=(ct == 0), stop=(ct == CT - 1))
            bnm = "b" + nm[1:]
            braw = wpool.tile([1, C], F32, name=f"{bnm}_raw", tag=f"{bnm}_raw")
            nc.sync.dma_start(out=braw, in_=b_d[bnm][:])
            bias_sb = wpool.tile([1, C], F32, name=f"{bnm}_sb", tag=f"{bnm}_sb")
            nc.vector.tensor_add(bias_sb, bias_ps, braw)
            if qscale is not None:
                nc.vector.tensor_scalar_mul(bias_sb, bias_sb, qscale)
            b_pm = []
            for ct in range(CT):
                bp_ps = pspool.tile([P, 1], F32, name=f"{bnm}_pps", tag="bp_ps")
                nc.tensor.matmul(bp_ps, lhsT=bias_sb[0:1, ct * P:(ct + 1) * P],
                                 rhs=one11, start=True, stop=True)
                bp = pmisc.tile([P, 1], F32, name=f"{bnm}4_{ct}")
                nc.vector.tensor_copy(bp, bp_ps)
                b_pm.append(bp)
            return wf, b_pm

        def transpose_chunk(src_tiles, xt_tiles, pst):
            """4 natural [128, C] pixel tiles -> x^T chunk tiles [P, CHUNK]."""
            for ct in range(CT):
                for i in range(4):
                    tp = pst.tile([P, P], src_tiles[i].dtype, name="tp", tag="tp")
                    nc.tensor.matmul(tp, lhsT=src_tiles[i][:, ct * P:(ct + 1) * P],
                                     rhs=ident, is_transpose=True)
                    nc.scalar.copy(xt_tiles[ct][:, i * P:(i + 1) * P], tp)

        with tc.tile_pool(name="pw1", bufs=1) as pw1, \
             tc.tile_pool(name="pwraw", bufs=2) as pwraw:
            with tc.tile_pool(name="psc", bufs=2, space="PSUM") as psc:
                wk_f, bk4 = fold_weight("wk", a4, None, pw1, psc, pwraw)
                wq_f, bq4 = fold_weight("wq", aq4, 1.0 / float(np.sqrt(C)), pw1, psc, pwraw)

            with tc.tile_pool(name="pxa", bufs=2) as pxa, \
                 tc.tile_pool(name="pxt", bufs=2) as pxt, \
                 tc.tile_pool(name="pst", bufs=4, space="PSUM") as pst, \
                 tc.tile_pool(name="psp", bufs=2, space="PSUM") as psp:
                # K projection over the full image, 512-pixel chunks
                for ch in range(NCH):
                    xa_t = []
                    for i in range(4):
                        xa = pxa.tile([P, C], xdt, name="xd1", tag=f"xd1_{i}")
                        nc.sync.dma_start(
                            out=xa, in_=x_full[(ch * 4 + i) * P:(ch * 4 + i + 1) * P, :])
                        xa_t.append(xa)
                    xt_t = [pxt.tile([P, CHUNK], F32R, name="xt1", tag=f"xt1_{ct}")
                            for ct in range(CT)]
                    transpose_chunk(xa_t, xt_t, pst)
                    for co in range(CT):
                        kps = psp.tile([P, CHUNK], F32, name="kps", tag="kps")
                        for ct in range(CT):
                            nc.tensor.matmul(kps, lhsT=wk_f[ct][:, co * P:(co + 1) * P],
                                             rhs=xt_t[ct], start=(ct == 0),
                                             stop=(ct == CT - 1))
                        nc.scalar.activation(kt[co][:, ch * CHUNK:(ch + 1) * CHUNK],
                                             kps, AF.Identity, bias=bk4[co], scale=1.0)
                # Q projection over this core's quarter (from x_resr)
                for ch in range(QCH):
                    xa_t = []
                    for i in range(4):
                        xa = pxa.tile([P, C], xdt, name="xd1q", tag=f"xd1_{i}")
                        nc.sync.dma_start(
                            out=xa, in_=x_resr[(ch * 4 + i) * P:(ch * 4 + i + 1) * P, :])
                        xa_t.append(xa)
                    xt_t = [pxt.tile([P, CHUNK], F32R, name="xt1q", tag=f"xt1_{ct}")
                            for ct in range(CT)]
                    transpose_chunk(xa_t, xt_t, pst)
                    for co in range(CT):
                        qps = psp.tile([P, CHUNK], F32, name="qps", tag="kps")
                        for ct in range(CT):
                            nc.tensor.matmul(qps, lhsT=wq_f[ct][:, co * P:(co + 1) * P],
                                             rhs=xt_t[ct], start=(ct == 0),
                                             stop=(ct == CT - 1))
                        nc.scalar.activation(qt[co][:, ch * CHUNK:(ch + 1) * CHUNK],
                                             qps, AF.Identity, bias=bq4[co], scale=1.0)

        # ================= Phase C2+D2: fold wv, project V =================
        with tc.tile_pool(name="pw2", bufs=1) as pw2, \
             tc.tile_pool(name="pwraw2", bufs=2) as pwraw2:
          with tc.tile_pool(name="psc2", bufs=2, space="PSUM") as psc2:
            wv_f, _bv4 = fold_weight("wv", a4, None, pw2, psc2, pwraw2)
            # bv' must be added along the free dim -> broadcast to [P, C]
            bv_sb = pw2.tile([1, C], F32, name="bv_fold", tag="bv_fold")
            bvp_ps = psc2.tile([1, C], F32, name="bv_fps", tag="bias_ps2")
            # recompute w^T b for v from the f32r-scaled tiles is wrong
            # (needs raw w); fold_weight already produced partition-major
            # bv4, but we need the free-major [1, C] version: rebuild it.
            # Instead: bv_sb = sum_ct b4[ct]^T @ raw is gone; use matmul on
            # folded-unscaled? -> simplest: redo with fresh raw DMAs.
            raws = []
            for ct in range(CT):
                raw = pwraw2.tile([P, C], F32, name="wv_raw2", tag="wv_raw2b")
                nc.sync.dma_start(out=raw, in_=w_d["wv"][ct * P:(ct + 1) * P, :])
                raws.append(raw)
            for ct in range(CT):
                nc.tensor.matmul(bvp_ps, lhsT=b4[ct], rhs=raws[ct],
                                 start=(ct == 0), stop=(ct == CT - 1))
            braw = pwraw2.tile([1, C], F32, name="bv_raw2", tag="bv_raw2")
            nc.sync.dma_start(out=braw, in_=b_d["bv"][:])
            nc.vector.tensor_add(bv_sb, bvp_ps, braw)
            bv_b = pmisc.tile([P, C], F32, name="bv_b")
            nc.gpsimd.partition_broadcast(bv_b, bv_sb)

          if True:
            with tc.tile_pool(name="pxa2", bufs=2) as pxa2, \
                 tc.tile_pool(name="pxt2", bufs=2) as pxt2, \
                 tc.tile_pool(name="pst2", bufs=4, space="PSUM") as pst2, \
                 tc.tile_pool(name="psp2", bufs=2, space="PSUM") as psp2:
                for ch in range(NCH):
                    xa_t = []
                    for i in range(4):
                        xa = pxa2.tile([P, C], xdt, name="xd2", tag=f"xd2_{i}")
                        nc.sync.dma_start(
                            out=xa, in_=x_full[(ch * 4 + i) * P:(ch * 4 + i + 1) * P, :])
                        xa_t.append(xa)
                    xt_t = [pxt2.tile([P, CHUNK], F32R, name="xt2", tag=f"xt2_{ct}")
                            for ct in range(CT)]
                    transpose_chunk(xa_t, xt_t, pst2)
                    for i in range(4):
                        vps = psp2.tile([P, C], F32, name="vps", tag="vps")
                        for ct in range(CT):
                            nc.tensor.matmul(vps, lhsT=xt_t[ct][:, i * P:(i + 1) * P],
                                             rhs=wv_f[ct], start=(ct == 0),
                                             stop=(ct == CT - 1))
                        nc.vector.tensor_add(vv[ch * 4 + i], vps, bv_b)

        # ===================== Phase E/F: attention ========================
        with tc.tile_pool(name="pwo", bufs=1) as pwo, \
             tc.tile_pool(name="pres", bufs=1) as pres, \
             tc.tile_pool(name="pe", bufs=3) as pe, \
             tc.tile_pool(name="pef", bufs=2) as pef, \
             tc.tile_pool(name="pss", bufs=2, space="PSUM") as pss, \
             tc.tile_pool(name="psat", bufs=1, space="PSUM") as psat, \
             tc.tile_pool(name="psr", bufs=1, space="PSUM") as psr, \
             tc.tile_pool(name="pso", bufs=1, space="PSUM") as pso:
            wo_f = []
            for ct in range(CT):
                raw = pef.tile([P, C], F32, name="wo_raw", tag="wo_raw")
                nc.sync.dma_start(out=raw, in_=w_d["wo"][ct * P:(ct + 1) * P, :])
                wo_t = pwo.tile([P, C], F32R, name=f"wo_f{ct}", tag=f"wo_f{ct}")
                nc.scalar.copy(wo_t, raw)
                wo_f.append(wo_t)
            bo_raw = pef.tile([1, C], F32, name="bo_raw", tag="bo_raw")
            nc.sync.dma_start(out=bo_raw, in_=b_d["bo"][:])
            bo_b = pwo.tile([P, C], F32, name="bo_b", tag="bo_b")
            nc.gpsimd.partition_broadcast(bo_b, bo_raw)
            # residual + bo, precomputed per query tile
            resb = []
            for i in range(QTILES):
                rraw = pef.tile([P, C], F32, name="rraw", tag="rraw")
                nc.sync.dma_start(out=rraw, in_=x_res[i * P:(i + 1) * P, :])
                rb = pres.tile([P, C], F32, name=f"resb{i}", tag=f"resb{i}")
                nc.vector.tensor_add(rb, rraw, bo_b)
                resb.append(rb)

            at_ps = [psat.tile([P, CHUNK], F32, name=f"at{i}", tag=f"at{i}")
                     for i in range(CT)]
            for qc in range(QCH):
                qsl = qt  # [P, QS] tiles; slice per chunk below
                rows_ps = psr.tile([1, CHUNK], F32, name="rows", tag="rows")
                for kt_i in range(NT):
                    sc_ps = pss.tile([P, CHUNK], F32, name="sc", tag="sc")
                    for ct in range(CT):
                        nc.tensor.matmul(
                            sc_ps,
                            lhsT=kt[ct][:, kt_i * P:(kt_i + 1) * P],
                            rhs=qsl[ct][:, qc * CHUNK:(qc + 1) * CHUNK],
                            start=(ct == 0), stop=(ct == CT - 1))
                    probs = pe.tile([P, CHUNK], BF16, name="probs", tag="probs")
                    nc.scalar.activation(probs, sc_ps, AF.Exp)
                    for co in range(CT):
                        nc.tensor.matmul(
                            at_ps[co],
                            lhsT=vv[kt_i][:, co * P:(co + 1) * P],
                            rhs=probs,
                            start=(kt_i == 0), stop=(kt_i == NT - 1))
                    nc.tensor.matmul(rows_ps, lhsT=ones_bf, rhs=probs,
                                     start=(kt_i == 0), stop=(kt_i == NT - 1))
                recip = pe.tile([1, CHUNK], F32, name="recip", tag="recip")
                nc.vector.reciprocal(recip, rows_ps)
                recip_b = pe.tile([P, CHUNK], F32, name="recip_b", tag="recip_b")
                nc.gpsimd.partition_broadcast(recip_b, recip)
                at_sb = []
                for co in range(CT):
                    a_sb = pe.tile([P, CHUNK], F32R, name="at_sb", tag=f"at_sb{co}")
                    nc.vector.tensor_mul(a_sb, at_ps[co], recip_b)
                    at_sb.append(a_sb)
                for qi in range(4):
                    ops = pso.tile([P, C], F32, name="ops", tag="ops")
                    for ct in range(CT):
                        nc.tensor.matmul(ops, lhsT=at_sb[ct][:, qi * P:(qi + 1) * P],
                                         rhs=wo_f[ct], start=(ct == 0),
                                         stop=(ct == CT - 1))
                    fin = pe.tile([P, C], F32, name="fin", tag="fin")
                    nc.vector.tensor_add(fin, ops, resb[qc * 4 + qi])
                    r0 = (qc * 4 + qi) * P
                    nc.sync.dma_start(out=out_d[r0:r0 + P, :], in_=fin)

    nc.compile()
    return nc


def _consts():
    ident = np.eye(P, dtype=np.float32)
    gind = np.zeros((P, 8), dtype=np.float32)
    for p in range(P):
        gind[p, p // GS] = 1.0
    gindt = np.ascontiguousarray(gind.T)
    return ident, gind, gindt


def kernel(**inputs):
    global _NC_CACHE
    if _NC_CACHE is None:
        _NC_CACHE = _build()
    nc = _NC_CACHE

    x = np.ascontiguousarray(np.asarray(inputs["inputs"], dtype=np.float32))
    xf = x.reshape(B, N, C)
    ident, gind, gindt = _consts()
    shared = {
        "gamma": np.ascontiguousarray(np.asarray(inputs["gn_gamma"], np.float32)),
        "beta": np.ascontiguousarray(np.asarray(inputs["gn_beta"], np.float32)),
        "ident": ident, "gind": gind, "gindt": gindt,
    }
    for nm in ("wq", "wk", "wv", "wo"):
        shared[nm] = np.ascontiguousarray(np.asarray(inputs[nm], np.float32))
    for nm in ("bq", "bk", "bv", "bo"):
        shared[nm] = np.ascontiguousarray(np.asarray(inputs[nm], np.float32))

    in_maps = []
    for core in range(NCORES):
        b, qq = divmod(core, 4)
        xr = np.ascontiguousarray(xf[b, qq * QS:(qq + 1) * QS, :])
        m = dict(shared)
        m["x_full"] = np.ascontiguousarray(xf[b])
        m["x_res"] = xr
        m["x_resr"] = xr
        in_maps.append(m)

    res = run_bass_kernel_spmd(nc, in_maps, list(range(NCORES)))
    out = np.empty((B, N, C), dtype=np.float32)
    for core in range(NCORES):
        b, qq = divmod(core, 4)
        out[b, qq * QS:(qq + 1) * QS, :] = res.results[core]["out"]
    return out.reshape(B, HH, WW, C)


def _install_ntff_shim():
    """The agent image's antenv lacks axon_hooks; provide it so
    run_bass_kernel_spmd(trace=True) can NTFF-profile through axon."""
    import types
    import antenv
    if "antenv.axon_hooks" in sys.modules:
        return
    mod = types.ModuleType("antenv.axon_hooks")
    mod._hook = None
    def set_axon_ntff_profile_hook(h):
        mod._hook = h
    def get_axon_ntff_profile_hook():
        return mod._hook
    mod.set_axon_ntff_profile_hook = set_axon_ntff_profile_hook
    mod.get_axon_ntff_profile_hook = get_axon_ntff_profile_hook
    sys.modules["antenv.axon_hooks"] = mod
    antenv.axon_hooks = mod
    sys.path.insert(0, "/root/.axon_site")
    from trn_agent_boot.trn_boot import _ntff_profile_via_ctypes
    hook = _ntff_profile_via_ctypes("/opt/axon/libaxon_pjrt.so")
    set_axon_ntff_profile_hook(hook)


def run_traced(inputs, trace_kwargs=None):
    """Traced run for profiling: returns BassKernelResults with exec_time_ns."""
    global _NC_CACHE
    if _NC_CACHE is None:
        _NC_CACHE = _build()
    import tempfile
    x = np.ascontiguousarray(np.asarray(inputs["inputs"], dtype=np.float32))
    xf = x.reshape(B, N, C)
    ident, gind, gindt = _consts()
    shared = {
        "gamma": np.ascontiguousarray(np.asarray(inputs["gn_gamma"], np.float32)),
        "beta": np.ascontiguousarray(np.asarray(inputs["gn_beta"], np.float32)),
        "ident": ident, "gind": gind, "gindt": gindt,
    }
    for nm in ("wq", "wk", "wv", "wo", "bq", "bk", "bv", "bo"):
        shared[nm] = np.ascontiguousarray(np.asarray(inputs[nm], np.float32))
    in_maps = []
    for core in range(NCORES):
        b, qq = divmod(core, 4)
        xr = np.ascontiguousarray(xf[b, qq * QS:(qq + 1) * QS, :])
        m = dict(shared)
        m["x_full"] = np.ascontiguousarray(xf[b])
        m["x_res"] = xr
        m["x_resr"] = xr
        in_maps.append(m)
    _install_ntff_shim()
    tmpdir = tempfile.mkdtemp(prefix="trace_")
    res = run_bass_kernel_spmd(_NC_CACHE, in_maps, list(range(NCORES)),
                               trace=True, tmpdir=tmpdir,
                               trace_kwargs=trace_kwargs or {})
    return res, tmpdir


# revision 9
# speedup vs baseline: 1.0140x; 1.0140x over previous
"""Trainium2 Bass kernel for Conv2D (1x1) multi-head attention block.

Reference computation (per batch image of [64, 64, 512] = [N=4096, C=512]):
    x  = GroupNorm(inputs, G=32, eps=1e-6) * gamma + beta
    q, k, v = x @ wq + bq, x @ wk + bk, x @ wv + bv      (1x1 convs)
    scores  = (q / sqrt(C)) @ k^T                         [N, N]
    out     = softmax(scores) @ v @ wo + bo + inputs

Sharding: 8 cores = 2 batches x 4 query-quarters. Each core holds the full
image of its batch (needed for GroupNorm stats and full-attention K/V) and
computes the output rows of its query quarter only.  No collectives: the
redundant K/V compute is cheaper than a DRAM-bounce AllGather here.

Key implementation choices:
  - GroupNorm is folded into the projection weights: with per-channel
    a[c] = gamma*rstd, b[c] = beta - mean*gamma*rstd, we have
    K^T = (diag(a) wk)^T x^T + (wk^T b + bk) 1^T, so normalized
    activations are never materialized.  Stats come from ones-matmuls
    (per-channel sum / sum-of-squares) in float32r (TF32-like) during the
    single streaming pass over x.
  - x^T is produced once by PE transposes of 128x128 blocks and kept
    resident in bf16; K^T, Q^T, V and all attention matmuls run in bf16
    (fp32 PSUM accumulation).  bf16 weights get fast-weight-load, which
    roughly halves the per-matmul cost vs 4-byte dtypes.  The residual add
    and all softmax normalization stay fp32, and the attention output is
    only ~4% of the output magnitude, so end-to-end error stays ~2e-4.
  - Attention uses the transposed-scores layout: scores^T[k, q] tiles come
    from matmul(lhsT=K^T tile, rhs=Q^T chunk); exp runs on the scalar
    engine PSUM->SBUF (no max-subtraction: scores are O(1) by construction
    since q is pre-scaled by 1/sqrt(C)); probs^T feeds
    matmul(lhsT=V tile, rhs=probs^T) accumulating attn^T[c, q] in PSUM over
    all 32 key tiles, and a ones-column matmul accumulates the softmax
    denominators.  The output projection consumes the *unnormalized*
    attn^T immediately; 1/rowsum is applied per-partition at the final
    PSUM->SBUF copy, keeping the PE free of the softmax epilogue.
"""

import sys

sys.path.insert(0, "/opt/trn_rl_repo")

from contextlib import ExitStack

import numpy as np

import concourse.bacc as bacc
import concourse.tile as tile
from concourse import mybir
from concourse.bass_utils import run_bass_kernel_spmd

# Problem shape (hardcoded; kernel.py must be self-contained).
B, HH, WW, C = 2, 64, 64, 512
N = HH * WW          # 4096 pixels per batch image
G = 32               # groupnorm groups
GS = C // G          # 16 channels per group
EPS = 1e-6
P = 128              # partitions
CT = C // P          # 4 channel tiles
NT = N // P          # 32 pixel tiles per image
CHUNK = 512          # free-dim chunk for moving operands
NCH = N // CHUNK     # 8 pixel chunks per image
NCORES = 8
QS = N // 4          # 1024 query rows per core
QTILES = QS // P     # 8 query tiles per core
QCH = QS // CHUNK    # 2 query chunks per core
GROUP_COUNT = N * GS  # elements per (batch, group) for the mean/var

F32 = mybir.dt.float32
F32R = mybir.dt.float32r
BF16 = mybir.dt.bfloat16
AF = mybir.ActivationFunctionType

_NC_CACHE = None


def _build():
    nc = bacc.Bacc(None, target_bir_lowering=False, debug=False)

    # x is DMA'd into float32r tiles directly: the PE truncates f32r reads
    # internally, and only the stats matmuls consume x at f32r.
    x_full = nc.dram_tensor("x_full", [N, C], F32R, kind="ExternalInput")
    x_res = nc.dram_tensor("x_res", [QS, C], F32, kind="ExternalInput")
    x_resr = nc.dram_tensor("x_resr", [QS, C], F32R, kind="ExternalInput")
    gamma_d = nc.dram_tensor("gamma", [C], F32, kind="ExternalInput")
    beta_d = nc.dram_tensor("beta", [C], F32, kind="ExternalInput")
    w_d = {}
    b_d = {}
    for nm in ("wq", "wk", "wv", "wo"):
        w_d[nm] = nc.dram_tensor(nm, [C, C], F32, kind="ExternalInput")
    for nm in ("bq", "bk", "bv", "bo"):
        b_d[nm] = nc.dram_tensor(nm, [C], F32, kind="ExternalInput")
    ident_d = nc.dram_tensor("ident", [P, P], F32R, kind="ExternalInput")
    gind_d = nc.dram_tensor("gind", [P, 8], F32, kind="ExternalInput")
    gindt_d = nc.dram_tensor("gindt", [8, P], F32, kind="ExternalInput")
    out_d = nc.dram_tensor("out", [QS, C], F32, kind="ExternalOutput")

    with tile.TileContext(nc) as tc, ExitStack() as top:
        # ---- persistent pools ----
        consts = top.enter_context(tc.tile_pool(name="consts", bufs=1))
        pkt = top.enter_context(tc.tile_pool(name="pkt", bufs=1))
        pqt = top.enter_context(tc.tile_pool(name="pqt", bufs=1))
        pv = top.enter_context(tc.tile_pool(name="pv", bufs=1))
        pxt = top.enter_context(tc.tile_pool(name="pxt", bufs=1))
        pmisc = top.enter_context(tc.tile_pool(name="pmisc", bufs=1))

        ident = consts.tile([P, P], F32R, name="ident")
        nc.sync.dma_start(out=ident, in_=ident_d[:])
        gind = consts.tile([P, 8], F32, name="gind")
        nc.sync.dma_start(out=gind, in_=gind_d[:])
        gindt = consts.tile([8, P], F32, name="gindt")
        nc.sync.dma_start(out=gindt, in_=gindt_d[:])
        ones_f32 = consts.tile([P, 1], F32, name="ones_f32")
        nc.vector.memset(ones_f32, 1.0)
        ones_r = consts.tile([P, 1], F32R, name="ones_r")
        nc.scalar.copy(ones_r, ones_f32)
        ones_bf = consts.tile([P, 1], BF16, name="ones_bf")
        nc.scalar.copy(ones_bf, ones_f32)
        one11 = ones_f32[0:1, 0:1]

        gamma4, beta4 = [], []
        for ct in range(CT):
            gt_ = consts.tile([P, 1], F32, name=f"gamma4_{ct}")
            nc.sync.dma_start(out=gt_, in_=gamma_d[ct * P:(ct + 1) * P])
            gamma4.append(gt_)
            bt_ = consts.tile([P, 1], F32, name=f"beta4_{ct}")
            nc.sync.dma_start(out=bt_, in_=beta_d[ct * P:(ct + 1) * P])
            beta4.append(bt_)

        # Resident activations: x^T, K^T, Q^T, V natural -- all bf16
        xt = [pxt.tile([P, N], BF16, name=f"xt{i}", tag=f"xt{i}") for i in range(CT)]
        kt = [pkt.tile([P, N], BF16, name=f"kt{i}", tag=f"kt{i}") for i in range(CT)]
        qt = [pqt.tile([P, QS], BF16, name=f"qt{i}", tag=f"qt{i}") for i in range(CT)]
        vv = [pv.tile([P, C], BF16, name=f"v{i}", tag=f"v{i}") for i in range(NT)]
        # x^T of the query quarter (for Q projection)
        xtq = [pxt.tile([P, QS], BF16, name=f"xtq{i}", tag=f"xtq{i}")
               for i in range(CT)]

        with ExitStack() as dphase:
            pxa = dphase.enter_context(tc.tile_pool(name="pxa", bufs=2))
            pa = dphase.enter_context(tc.tile_pool(name="pa", bufs=2))
            pst = dphase.enter_context(tc.tile_pool(name="pst", bufs=3, space="PSUM"))
            psp = dphase.enter_context(tc.tile_pool(name="psp", bufs=2, space="PSUM"))

            # ==== Phase A: stream x once; stats matmuls + transposes ====
            def stream_chunk(src_dram, row0, xt_dst, col0, stats):
                """DMA 4 pixel tiles, optionally feed stats, transpose into
                xt_dst[ct][:, col0:col0+512]."""
                xa_t = []
                for i in range(4):
                    xa = pxa.tile([P, C], F32R, name="xa", tag=f"xa{i}")
                    nc.sync.dma_start(
                        out=xa,
                        in_=src_dram[row0 + i * P:row0 + (i + 1) * P, :])
                    xa_t.append(xa)
                if stats is not None:
                    s_ps, sq_ps, first, last = stats
                    for i in range(4):
                        x2 = pa.tile([P, C], F32R, name="x2", tag="x2")
                        nc.vector.tensor_mul(x2, xa_t[i], xa_t[i])
                        nc.tensor.matmul(s_ps, lhsT=ones_r, rhs=xa_t[i],
                                         start=(first and i == 0),
                                         stop=(last and i == 3))
                        nc.tensor.matmul(sq_ps, lhsT=ones_r, rhs=x2,
                                         start=(first and i == 0),
                                         stop=(last and i == 3))
                for ct in range(CT):
                    tp = pst.tile([P, C], F32R, name="tp", tag="tp")
                    for i in range(4):
                        nc.tensor.matmul(tp[:, i * P:(i + 1) * P],
                                         lhsT=xa_t[i][:, ct * P:(ct + 1) * P],
                                         rhs=ident, is_transpose=True)
                    nc.vector.tensor_copy(xt_dst[ct][:, col0:col0 + CHUNK], tp)

            with tc.tile_pool(name="psa", bufs=1, space="PSUM") as psa:
                s_ps = psa.tile([1, C], F32, name="s_ps", tag="s_ps")
                sq_ps = psa.tile([1, C], F32, name="sq_ps", tag="sq_ps")
                for ch in range(NCH):
                    stats = (s_ps, sq_ps, ch == 0, ch == NCH - 1)
                    stream_chunk(x_full, ch * CHUNK, xt, ch * CHUNK, stats)
                # transpose the query quarter too (no stats: subset of x)
                for ch in range(QCH):
                    stream_chunk(x_resr, ch * CHUNK, xtq, ch * CHUNK, None)
                s_sb = pmisc.tile([1, C], F32, name="s_sb")
                nc.vector.tensor_copy(s_sb, s_ps)
                sq_sb = pmisc.tile([1, C], F32, name="sq_sb")
                nc.vector.tensor_copy(sq_sb, sq_ps)

            # ==== Phase B: group stats -> per-channel a, b (partition-major)
            a4, aq4, b4 = [], [], []
            with tc.tile_pool(name="psb", bufs=1, space="PSUM") as psb, \
                 tc.tile_pool(name="pb", bufs=2) as pb:
                for ct in range(CT):
                    st4 = psb.tile([P, 2], F32, name="st4", tag="st4")
                    nc.tensor.matmul(st4[:, 0:1],
                                     lhsT=s_sb[0:1, ct * P:(ct + 1) * P],
                                     rhs=one11, start=True, stop=True)
                    nc.tensor.matmul(st4[:, 1:2],
                                     lhsT=sq_sb[0:1, ct * P:(ct + 1) * P],
                                     rhs=one11, start=True, stop=True)
                    st4_sb = pb.tile([P, 2], F32, name="st4_sb", tag="st4_sb")
                    nc.vector.tensor_copy(st4_sb, st4)
                    grp_ps = psb.tile([8, 2], F32, name="grp_ps", tag="grp_ps")
                    nc.tensor.matmul(grp_ps, lhsT=gind, rhs=st4_sb,
                                     start=True, stop=True)
                    grp = pb.tile([8, 2], F32, name="grp", tag="grp")
                    nc.vector.tensor_scalar_mul(grp, grp_ps, 1.0 / GROUP_COUNT)
                    var = pb.tile([8, 1], F32, name="var", tag="var")
                    nc.vector.tensor_mul(var, grp[:, 0:1], grp[:, 0:1])
                    nc.vector.tensor_sub(var, grp[:, 1:2], var)
                    nc.vector.tensor_scalar_add(var, var, EPS)
                    rstd = pb.tile([8, 1], F32, name="rstd", tag="rstd")
                    nc.vector.reciprocal(rstd, var)
                    nc.scalar.sqrt(rstd, rstd)
                    mr = pb.tile([8, 2], F32, name="mr", tag="mr")
                    nc.vector.tensor_copy(mr[:, 0:1], grp[:, 0:1])
                    nc.vector.tensor_copy(mr[:, 1:2], rstd)
                    mch_ps = psb.tile([P, 2], F32, name="mch_ps", tag="mch_ps")
                    nc.tensor.matmul(mch_ps, lhsT=gindt, rhs=mr,
                                     start=True, stop=True)
                    mch = pb.tile([P, 2], F32, name="mch", tag="mch")
                    nc.vector.tensor_copy(mch, mch_ps)
                    a_t = pmisc.tile([P, 1], F32, name=f"a4_{ct}")
                    nc.vector.tensor_mul(a_t, gamma4[ct], mch[:, 1:2])
                    a4.append(a_t)
                    aq_t = pmisc.tile([P, 1], F32, name=f"aq4_{ct}")
                    nc.vector.tensor_scalar_mul(aq_t, a_t, 1.0 / float(np.sqrt(C)))
                    aq4.append(aq_t)
                    b_t = pmisc.tile([P, 1], F32, name=f"b4_{ct}")
                    nc.vector.tensor_mul(b_t, mch[:, 0:1], a_t)
                    nc.vector.tensor_sub(b_t, beta4[ct], b_t)
                    b4.append(b_t)

            # ==== Phase C: fold weights (bf16) + biases ====
            def fold_weight(nm, scales, qscale, pool, pspool, wpool):
                wf, raws = [], []
                for ct in range(CT):
                    raw = wpool.tile([P, C], F32, name=f"{nm}_raw",
                                     tag=f"{nm}_raw")
                    nc.sync.dma_start(out=raw,
                                      in_=w_d[nm][ct * P:(ct + 1) * P, :])
                    raws.append(raw)
                    wf_t = pool.tile([P, C], BF16, name=f"{nm}_f{ct}",
                                     tag=f"{nm}_f{ct}")
                    nc.scalar.mul(wf_t, raw, scales[ct])
                    wf.append(wf_t)
                bias_ps = pspool.tile([1, C], F32, name=f"{nm}_bps", tag="bias")
                for ct in range(CT):
                    nc.tensor.matmul(bias_ps, lhsT=b4[ct], rhs=raws[ct],
                                     start=(ct == 0), stop=(ct == CT - 1))
                bnm = "b" + nm[1:]
                braw = wpool.tile([1, C], F32, name=f"{bnm}_raw", tag="braw")
                nc.sync.dma_start(out=braw, in_=b_d[bnm][:])
                bias_sb = pmisc.tile([1, C], F32, name=f"{bnm}_sb")
                nc.vector.tensor_add(bias_sb, bias_ps, braw)
                if qscale is not None:
                    nc.vector.tensor_scalar_mul(bias_sb, bias_sb, qscale)
                b_pm = []
                for ct in range(CT):
                    bp_ps = pspool.tile([P, 1], F32, name=f"{bnm}_pps",
                                        tag="bias")
                    nc.tensor.matmul(bp_ps,
                                     lhsT=bias_sb[0:1, ct * P:(ct + 1) * P],
                                     rhs=one11, start=True, stop=True)
                    bp = pmisc.tile([P, 1], F32, name=f"{bnm}4_{ct}")
                    nc.vector.tensor_copy(bp, bp_ps)
                    b_pm.append(bp)
                return wf, bias_sb, b_pm

            with tc.tile_pool(name="pw", bufs=1) as pw, \
                 tc.tile_pool(name="pwraw", bufs=1) as pwraw, \
                 tc.tile_pool(name="psc", bufs=2, space="PSUM") as psc:
                wk_f, _, bk4 = fold_weight("wk", a4, None, pw, psc, pwraw)
                wq_f, _, bq4 = fold_weight(
                    "wq", aq4, 1.0 / float(np.sqrt(C)), pw, psc, pwraw)
                wv_f, bv_sb, _ = fold_weight("wv", a4, None, pw, psc, pwraw)
                bv_b = pmisc.tile([P, C], F32, name="bv_b")
                nc.gpsimd.partition_broadcast(bv_b, bv_sb)

                # ==== Phase D: projections from resident x^T ====
                # K^T[co][:, chunk] = sum_ct wk'[ct][:,co*128:] ^T @ x^T[ct]
                for ch in range(NCH):
                    for co in range(CT):
                        kps = psp.tile([P, CHUNK], F32, name="kps", tag="kps")
                        for ct in range(CT):
                            nc.tensor.matmul(
                                kps, lhsT=wk_f[ct][:, co * P:(co + 1) * P],
                                rhs=xt[ct][:, ch * CHUNK:(ch + 1) * CHUNK],
                                start=(ct == 0), stop=(ct == CT - 1))
                        nc.scalar.activation(
                            kt[co][:, ch * CHUNK:(ch + 1) * CHUNK], kps,
                            AF.Identity, bias=bk4[co], scale=1.0)
                for ch in range(QCH):
                    for co in range(CT):
                        qps = psp.tile([P, CHUNK], F32, name="qps", tag="kps")
                        for ct in range(CT):
                            nc.tensor.matmul(
                                qps, lhsT=wq_f[ct][:, co * P:(co + 1) * P],
                                rhs=xtq[ct][:, ch * CHUNK:(ch + 1) * CHUNK],
                                start=(ct == 0), stop=(ct == CT - 1))
                        nc.scalar.activation(
                            qt[co][:, ch * CHUNK:(ch + 1) * CHUNK], qps,
                            AF.Identity, bias=bq4[co], scale=1.0)
                # V natural: lhsT = x^T pixel-block, rhs = wv'
                for nt_i in range(NT):
                    vps = psp.tile([P, C], F32, name="vps", tag="kps")
                    for ct in range(CT):
                        nc.tensor.matmul(
                            vps, lhsT=xt[ct][:, nt_i * P:(nt_i + 1) * P],
                            rhs=wv_f[ct], start=(ct == 0), stop=(ct == CT - 1))
                    nc.vector.tensor_add(vv[nt_i], vps, bv_b)

        # ==== Phase E/F: attention + output projection ====
        with tc.tile_pool(name="pwo", bufs=1) as pwo, \
             tc.tile_pool(name="pres", bufs=1) as pres, \
             tc.tile_pool(name="pe", bufs=3) as pe, \
             tc.tile_pool(name="pef", bufs=2) as pef, \
             tc.tile_pool(name="pss", bufs=1, space="PSUM") as pss, \
             tc.tile_pool(name="psat", bufs=1, space="PSUM") as psat, \
             tc.tile_pool(name="psr", bufs=2, space="PSUM") as psr, \
             tc.tile_pool(name="pso", bufs=1, space="PSUM") as pso:
            wo_f = []
            for ct in range(CT):
                raw = pef.tile([P, C], F32, name="wo_raw", tag="wo_raw")
                nc.sync.dma_start(out=raw, in_=w_d["wo"][ct * P:(ct + 1) * P, :])
                wo_t = pwo.tile([P, C], BF16, name=f"wo_f{ct}", tag=f"wo_f{ct}")
                nc.scalar.copy(wo_t, raw)
                wo_f.append(wo_t)
            bo_raw = pef.tile([1, C], F32, name="bo_raw", tag="bo_raw")
            nc.sync.dma_start(out=bo_raw, in_=b_d["bo"][:])
            bo_b = pwo.tile([P, C], F32, name="bo_b", tag="bo_b")
            nc.gpsimd.partition_broadcast(bo_b, bo_raw)
            resb = []
            for i in range(QTILES):
                rraw = pef.tile([P, C], F32, name="rraw", tag="rraw")
                nc.sync.dma_start(out=rraw, in_=x_res[i * P:(i + 1) * P, :])
                rb = pres.tile([P, C], F32, name=f"resb{i}", tag=f"resb{i}")
                nc.vector.tensor_add(rb, rraw, bo_b)
                resb.append(rb)

            at_ps = [psat.tile([P, CHUNK], F32, name=f"at{i}", tag=f"at{i}")
                     for i in range(CT)]
            for qc in range(QCH):
                rows_ps = psr.tile([1, CHUNK], F32, name="rows", tag="rows")
                for kt_i in range(NT):
                    sc_ps = pss.tile([P, CHUNK], F32, name="sc", tag="sc")
                    for ct in range(CT):
                        nc.tensor.matmul(
                            sc_ps,
                            lhsT=kt[ct][:, kt_i * P:(kt_i + 1) * P],
                            rhs=qt[ct][:, qc * CHUNK:(qc + 1) * CHUNK],
                            start=(ct == 0), stop=(ct == CT - 1))
                    probs = pe.tile([P, CHUNK], BF16, name="probs", tag="probs")
                    nc.scalar.activation(probs, sc_ps, AF.Exp)
                    for co in range(CT):
                        nc.tensor.matmul(
                            at_ps[co],
                            lhsT=vv[kt_i][:, co * P:(co + 1) * P],
                            rhs=probs,
                            start=(kt_i == 0), stop=(kt_i == NT - 1))
                    nc.tensor.matmul(rows_ps, lhsT=ones_bf, rhs=probs,
                                     start=(kt_i == 0), stop=(kt_i == NT - 1))
                # softmax denominators -> per-partition reciprocals
                recip = pe.tile([1, CHUNK], F32, name="recip", tag="recip")
                nc.vector.reciprocal(recip, rows_ps)
                recip4 = []
                for qi in range(4):
                    r4_ps = psr.tile([P, 1], F32, name="r4", tag="rows")
                    nc.tensor.matmul(r4_ps,
                                     lhsT=recip[0:1, qi * P:(qi + 1) * P],
                                     rhs=one11, start=True, stop=True)
                    r4 = pe.tile([P, 1], F32, name="recip4", tag=f"recip4_{qi}")
                    nc.vector.tensor_copy(r4, r4_ps)
                    recip4.append(r4)
                # unnormalized attn^T -> SBUF (no dependency on rowsums)
                at_sb = []
                for co in range(CT):
                    a_sb = pe.tile([P, CHUNK], BF16, name="at_sb",
                                   tag=f"at_sb{co}")
                    nc.scalar.copy(a_sb, at_ps[co])
                    at_sb.append(a_sb)
                for qi in range(4):
                    ops = pso.tile([P, C], F32, name="ops", tag="ops")
                    for ct in range(CT):
                        nc.tensor.matmul(
                            ops, lhsT=at_sb[ct][:, qi * P:(qi + 1) * P],
                            rhs=wo_f[ct], start=(ct == 0), stop=(ct == CT - 1))
                    fin = pe.tile([P, C], F32, name="fin", tag="fin")
                    # normalize rows here: out_row *= 1/rowsum (per-partition)
                    nc.scalar.activation(fin, ops, AF.Copy, bias=0.0,
                                         scale=recip4[qi])
                    fin2 = pe.tile([P, C], F32, name="fin2", tag="fin2")
                    nc.vector.tensor_add(fin2, fin, resb[qc * 4 + qi])
                    r0 = (qc * 4 + qi) * P
                    nc.sync.dma_start(out=out_d[r0:r0 + P, :], in_=fin2)

    nc.compile()
    return nc


def _consts():
    ident = np.eye(P, dtype=np.float32)
    gind = np.zeros((P, 8), dtype=np.float32)
    for p in range(P):
        gind[p, p // GS] = 1.0
    gindt = np.ascontiguousarray(gind.T)
    return ident, gind, gindt


def _make_in_maps(inputs):
    x = np.ascontiguousarray(np.asarray(inputs["inputs"], dtype=np.float32))
    xf = x.reshape(B, N, C)
    ident, gind, gindt = _consts()
    shared = {
        "gamma": np.ascontiguousarray(np.asarray(inputs["gn_gamma"], np.float32)),
        "beta": np.ascontiguousarray(np.asarray(inputs["gn_beta"], np.float32)),
        "ident": ident, "gind": gind, "gindt": gindt,
    }
    for nm in ("wq", "wk", "wv", "wo", "bq", "bk", "bv", "bo"):
        shared[nm] = np.ascontiguousarray(np.asarray(inputs[nm], np.float32))
    in_maps = []
    for core in range(NCORES):
        b, qq = divmod(core, 4)
        xr = np.ascontiguousarray(xf[b, qq * QS:(qq + 1) * QS, :])
        m = dict(shared)
        m["x_full"] = np.ascontiguousarray(xf[b])
        m["x_res"] = xr
        m["x_resr"] = xr
        in_maps.append(m)
    return in_maps


def _assemble(results):
    out = np.empty((B, N, C), dtype=np.float32)
    for core in range(NCORES):
        b, qq = divmod(core, 4)
        out[b, qq * QS:(qq + 1) * QS, :] = results[core]["out"]
    return out.reshape(B, HH, WW, C)


def kernel(**inputs):
    global _NC_CACHE
    if _NC_CACHE is None:
        _NC_CACHE = _build()
    in_maps = _make_in_maps(inputs)
    res = run_bass_kernel_spmd(_NC_CACHE, in_maps, list(range(NCORES)))
    return _assemble(res.results)


def _install_ntff_shim():
    """The agent image's antenv lacks axon_hooks; provide it so
    run_bass_kernel_spmd(trace=True) can NTFF-profile through axon."""
    import types
    import antenv
    if "antenv.axon_hooks" in sys.modules:
        return
    mod = types.ModuleType("antenv.axon_hooks")
    mod._hook = None

    def set_axon_ntff_profile_hook(h):
        mod._hook = h

    def get_axon_ntff_profile_hook():
        return mod._hook

    mod.set_axon_ntff_profile_hook = set_axon_ntff_profile_hook
    mod.get_axon_ntff_profile_hook = get_axon_ntff_profile_hook
    sys.modules["antenv.axon_hooks"] = mod
    antenv.axon_hooks = mod
    sys.path.insert(0, "/root/.axon_site")
    from trn_agent_boot.trn_boot import _ntff_profile_via_ctypes
    hook = _ntff_profile_via_ctypes("/opt/axon/libaxon_pjrt.so")
    set_axon_ntff_profile_hook(hook)


def run_traced(inputs, trace_kwargs=None):
    """Traced run for profiling: returns (BassKernelResults, tmpdir)."""
    global _NC_CACHE
    if _NC_CACHE is None:
        _NC_CACHE = _build()
    import tempfile
    _install_ntff_shim()
    in_maps = _make_in_maps(inputs)
    tmpdir = tempfile.mkdtemp(prefix="trace_")
    res = run_bass_kernel_spmd(_NC_CACHE, in_maps, list(range(NCORES)),
                               trace=True, tmpdir=tmpdir,
                               trace_kwargs=trace_kwargs or {})
    return res, tmpdir


# revision 10
# speedup vs baseline: 1.2181x; 1.2013x over previous
"""Trainium2 Bass kernel for Conv2D (1x1) multi-head attention block.

Reference computation (per batch image of [64, 64, 512] = [N=4096, C=512]):
    x  = GroupNorm(inputs, G=32, eps=1e-6) * gamma + beta
    q, k, v = x @ wq + bq, x @ wk + bk, x @ wv + bv      (1x1 convs)
    scores  = (q / sqrt(C)) @ k^T                         [N, N]
    out     = softmax(scores) @ v @ wo + bo + inputs

Sharding: 8 cores = 2 batches x 4 query-quarters. Each core holds the full
image of its batch (needed for GroupNorm stats and full-attention K/V) and
computes the output rows of its query quarter only.  No collectives: the
redundant K/V compute is cheaper than a DRAM-bounce AllGather here.

Key implementation choices:
  - GroupNorm is folded into the projection weights: with per-channel
    a[c] = gamma*rstd, b[c] = beta - mean*gamma*rstd, we have
    K^T = (diag(a) wk)^T x^T + (wk^T b + bk) 1^T, so normalized
    activations are never materialized.  Stats come from ones-matmuls
    (per-channel sum / sum-of-squares) in float32r (TF32-like) during the
    single streaming pass over x.
  - x^T is produced once by PE transposes of 128x128 blocks and kept
    resident in bf16; K^T, Q^T, V and all attention matmuls run in bf16
    (fp32 PSUM accumulation).  bf16 weights get fast-weight-load, which
    roughly halves the per-matmul cost vs 4-byte dtypes.  The residual add
    and all softmax normalization stay fp32, and the attention output is
    only ~4% of the output magnitude, so end-to-end error stays ~2e-4.
  - Attention uses the transposed-scores layout: scores^T[k, q] tiles come
    from matmul(lhsT=K^T tile, rhs=Q^T chunk); exp runs on the scalar
    engine PSUM->SBUF (no max-subtraction: scores are O(1) by construction
    since q is pre-scaled by 1/sqrt(C)); probs^T feeds
    matmul(lhsT=V tile, rhs=probs^T) accumulating attn^T[c, q] in PSUM over
    all 32 key tiles, and a ones-column matmul accumulates the softmax
    denominators.  The output projection consumes the *unnormalized*
    attn^T immediately; 1/rowsum is applied per-partition at the final
    PSUM->SBUF copy, keeping the PE free of the softmax epilogue.
"""

import sys

sys.path.insert(0, "/opt/trn_rl_repo")

from contextlib import ExitStack

import numpy as np

import concourse.bacc as bacc
import concourse.tile as tile
from concourse import mybir
from concourse.bass_utils import run_bass_kernel_spmd

# Problem shape (hardcoded; kernel.py must be self-contained).
B, HH, WW, C = 2, 64, 64, 512
N = HH * WW          # 4096 pixels per batch image
G = 32               # groupnorm groups
GS = C // G          # 16 channels per group
EPS = 1e-6
P = 128              # partitions
CT = C // P          # 4 channel tiles
NT = N // P          # 32 pixel tiles per image
CHUNK = 512          # free-dim chunk for moving operands
NCH = N // CHUNK     # 8 pixel chunks per image
NCORES = 8
QS = N // 4          # 1024 query rows per core
QTILES = QS // P     # 8 query tiles per core
QCH = QS // CHUNK    # 2 query chunks per core
GROUP_COUNT = N * GS  # elements per (batch, group) for the mean/var

F32 = mybir.dt.float32
F32R = mybir.dt.float32r
BF16 = mybir.dt.bfloat16
AF = mybir.ActivationFunctionType

_NC_CACHE = None


def _build():
    nc = bacc.Bacc(None, target_bir_lowering=False, debug=False)

    # x is DMA'd into float32r tiles directly: the PE truncates f32r reads
    # internally, and only the stats matmuls consume x at f32r.
    x_full = nc.dram_tensor("x_full", [N, C], F32R, kind="ExternalInput")
    x_res = nc.dram_tensor("x_res", [QS, C], F32, kind="ExternalInput")
    x_resr = nc.dram_tensor("x_resr", [QS, C], F32R, kind="ExternalInput")
    gamma_d = nc.dram_tensor("gamma", [C], F32, kind="ExternalInput")
    beta_d = nc.dram_tensor("beta", [C], F32, kind="ExternalInput")
    w_d = {}
    b_d = {}
    for nm in ("wq", "wk", "wv", "wo"):
        w_d[nm] = nc.dram_tensor(nm, [C, C], F32, kind="ExternalInput")
    for nm in ("bq", "bk", "bv", "bo"):
        b_d[nm] = nc.dram_tensor(nm, [C], F32, kind="ExternalInput")
    ident_d = nc.dram_tensor("ident", [P, P], F32R, kind="ExternalInput")
    gind_d = nc.dram_tensor("gind", [P, 8], F32, kind="ExternalInput")
    gindt_d = nc.dram_tensor("gindt", [8, P], F32, kind="ExternalInput")
    out_d = nc.dram_tensor("out", [QS, C], F32, kind="ExternalOutput")

    with tile.TileContext(nc) as tc, ExitStack() as top:
        # ---- persistent pools ----
        consts = top.enter_context(tc.tile_pool(name="consts", bufs=1))
        pkt = top.enter_context(tc.tile_pool(name="pkt", bufs=1))
        pqt = top.enter_context(tc.tile_pool(name="pqt", bufs=1))
        pv = top.enter_context(tc.tile_pool(name="pv", bufs=1))
        pxt = top.enter_context(tc.tile_pool(name="pxt", bufs=1))
        pmisc = top.enter_context(tc.tile_pool(name="pmisc", bufs=1))

        ident = consts.tile([P, P], F32R, name="ident")
        nc.sync.dma_start(out=ident, in_=ident_d[:])
        gind = consts.tile([P, 8], F32, name="gind")
        nc.sync.dma_start(out=gind, in_=gind_d[:])
        gindt = consts.tile([8, P], F32, name="gindt")
        nc.sync.dma_start(out=gindt, in_=gindt_d[:])
        ones_f32 = consts.tile([P, 1], F32, name="ones_f32")
        nc.vector.memset(ones_f32, 1.0)
        ones_bf = consts.tile([P, 1], BF16, name="ones_bf")
        nc.scalar.copy(ones_bf, ones_f32)
        one11 = ones_f32[0:1, 0:1]

        gamma4, beta4 = [], []
        for ct in range(CT):
            gt_ = consts.tile([P, 1], F32, name=f"gamma4_{ct}")
            nc.sync.dma_start(out=gt_, in_=gamma_d[ct * P:(ct + 1) * P])
            gamma4.append(gt_)
            bt_ = consts.tile([P, 1], F32, name=f"beta4_{ct}")
            nc.sync.dma_start(out=bt_, in_=beta_d[ct * P:(ct + 1) * P])
            beta4.append(bt_)

        # Resident activations: x^T, K^T, Q^T, V natural -- all bf16
        xt = [pxt.tile([P, N], BF16, name=f"xt{i}", tag=f"xt{i}") for i in range(CT)]
        kt = [pkt.tile([P, N], BF16, name=f"kt{i}", tag=f"kt{i}") for i in range(CT)]
        qt = [pqt.tile([P, QS], BF16, name=f"qt{i}", tag=f"qt{i}") for i in range(CT)]
        vv = [pv.tile([P, C], BF16, name=f"v{i}", tag=f"v{i}") for i in range(NT)]
        # x^T of the query quarter (for Q projection)
        xtq = [pxt.tile([P, QS], BF16, name=f"xtq{i}", tag=f"xtq{i}")
               for i in range(CT)]

        with ExitStack() as dphase:
            pxa = dphase.enter_context(tc.tile_pool(name="pxa", bufs=2))
            pst = dphase.enter_context(tc.tile_pool(name="pst", bufs=3, space="PSUM"))
            psp = dphase.enter_context(tc.tile_pool(name="psp", bufs=2, space="PSUM"))

            # per-channel bn_stats accumulators, one [P, NCH, 6] per ct
            bnst = [pmisc.tile([P, NCH, 6], F32, name=f"bnst{i}")
                    for i in range(CT)]

            # ==== Phase A: stream x once; transposes + bn_stats on x^T ====
            def stream_chunk(src_dram, row0, xt_dst, col0, ch_idx):
                """DMA 4 pixel tiles, transpose into xt_dst[ct][:, col0:],
                and (for x_full chunks) fold per-channel bn_stats."""
                xa_t = []
                for i in range(4):
                    xa = pxa.tile([P, C], F32R, name="xa", tag=f"xa{i}")
                    nc.sync.dma_start(
                        out=xa,
                        in_=src_dram[row0 + i * P:row0 + (i + 1) * P, :])
                    xa_t.append(xa)
                for ct in range(CT):
                    tp = pst.tile([P, C], F32R, name="tp", tag="tp")
                    for i in range(4):
                        nc.tensor.matmul(tp[:, i * P:(i + 1) * P],
                                         lhsT=xa_t[i][:, ct * P:(ct + 1) * P],
                                         rhs=ident, is_transpose=True)
                    nc.vector.tensor_copy(xt_dst[ct][:, col0:col0 + CHUNK], tp)
                    if ch_idx is not None:
                        nc.vector.bn_stats(
                            out=bnst[ct][:, ch_idx, :],
                            in_=xt_dst[ct][:, col0:col0 + CHUNK])

            for ch in range(NCH):
                stream_chunk(x_full, ch * CHUNK, xt, ch * CHUNK, ch)
            # transpose the query quarter too (no stats: subset of x)
            for ch in range(QCH):
                stream_chunk(x_resr, ch * CHUNK, xtq, ch * CHUNK, None)

            # Warm-keeper: idle-PE filler matmuls so the HAM clock gate
            # stays at full rate through the serial stats/fold section.
            warm32 = pmisc.tile([P, CHUNK], F32, name="warm32")
            nc.vector.memset(warm32, 1.0)
            warm_src = pmisc.tile([P, CHUNK], F32R, name="warm_src")
            nc.scalar.copy(warm_src, warm32)

            def keep_warm(n):
                for _ in range(n):
                    wps = psp.tile([P, CHUNK], F32, name="wps", tag="kps")
                    nc.tensor.matmul(wps, lhsT=ident, rhs=warm_src,
                                     start=True, stop=True)

            # ==== Phase B: group stats -> per-channel a, b (partition-major)
            a4, aq4, b4 = [], [], []
            with tc.tile_pool(name="psb", bufs=1, space="PSUM") as psb, \
                 tc.tile_pool(name="pb", bufs=2) as pb:
                for ct in range(CT):
                    mv = pb.tile([P, 2], F32, name="mv", tag="mv")
                    nc.vector.bn_aggr(out=mv, in_=bnst[ct])
                    # per-channel (mean, E[x^2])
                    me2 = pb.tile([P, 2], F32, name="me2", tag="me2")
                    nc.vector.tensor_copy(me2[:, 0:1], mv[:, 0:1])
                    nc.vector.tensor_mul(me2[:, 1:2], mv[:, 0:1], mv[:, 0:1])
                    nc.vector.tensor_add(me2[:, 1:2], me2[:, 1:2], mv[:, 1:2])
                    keep_warm(6)
                    grp_ps = psb.tile([8, 2], F32, name="grp_ps", tag="grp_ps")
                    nc.tensor.matmul(grp_ps, lhsT=gind, rhs=me2,
                                     start=True, stop=True)
                    grp = pb.tile([8, 2], F32, name="grp", tag="grp")
                    nc.vector.tensor_scalar_mul(grp, grp_ps, 1.0 / GS)
                    var = pb.tile([8, 1], F32, name="var", tag="var")
                    nc.vector.tensor_mul(var, grp[:, 0:1], grp[:, 0:1])
                    nc.vector.tensor_sub(var, grp[:, 1:2], var)
                    nc.vector.tensor_scalar_add(var, var, EPS)
                    rstd = pb.tile([8, 1], F32, name="rstd", tag="rstd")
                    nc.vector.reciprocal(rstd, var)
                    nc.scalar.sqrt(rstd, rstd)
                    mr = pb.tile([8, 2], F32, name="mr", tag="mr")
                    nc.vector.tensor_copy(mr[:, 0:1], grp[:, 0:1])
                    nc.vector.tensor_copy(mr[:, 1:2], rstd)
                    mch_ps = psb.tile([P, 2], F32, name="mch_ps", tag="mch_ps")
                    nc.tensor.matmul(mch_ps, lhsT=gindt, rhs=mr,
                                     start=True, stop=True)
                    keep_warm(6)
                    mch = pb.tile([P, 2], F32, name="mch", tag="mch")
                    nc.vector.tensor_copy(mch, mch_ps)
                    a_t = pmisc.tile([P, 1], F32, name=f"a4_{ct}")
                    nc.vector.tensor_mul(a_t, gamma4[ct], mch[:, 1:2])
                    a4.append(a_t)
                    aq_t = pmisc.tile([P, 1], F32, name=f"aq4_{ct}")
                    nc.vector.tensor_scalar_mul(aq_t, a_t, 1.0 / float(np.sqrt(C)))
                    aq4.append(aq_t)
                    b_t = pmisc.tile([P, 1], F32, name=f"b4_{ct}")
                    nc.vector.tensor_mul(b_t, mch[:, 0:1], a_t)
                    nc.vector.tensor_sub(b_t, beta4[ct], b_t)
                    b4.append(b_t)

            # ==== Phase C: fold weights (bf16) + biases ====
            def fold_weight(nm, scales, qscale, pool, pspool, wpool):
                wf, raws = [], []
                for ct in range(CT):
                    raw = wpool.tile([P, C], F32, name=f"{nm}_raw",
                                     tag=f"{nm}_raw")
                    nc.sync.dma_start(out=raw,
                                      in_=w_d[nm][ct * P:(ct + 1) * P, :])
                    raws.append(raw)
                    wf_t = pool.tile([P, C], BF16, name=f"{nm}_f{ct}",
                                     tag=f"{nm}_f{ct}")
                    nc.scalar.mul(wf_t, raw, scales[ct])
                    wf.append(wf_t)
                keep_warm(8)
                bias_ps = pspool.tile([1, C], F32, name=f"{nm}_bps", tag="bias")
                for ct in range(CT):
                    nc.tensor.matmul(bias_ps, lhsT=b4[ct], rhs=raws[ct],
                                     start=(ct == 0), stop=(ct == CT - 1))
                bnm = "b" + nm[1:]
                braw = wpool.tile([1, C], F32, name=f"{bnm}_raw", tag="braw")
                nc.sync.dma_start(out=braw, in_=b_d[bnm][:])
                bias_sb = pmisc.tile([1, C], F32, name=f"{bnm}_sb")
                nc.vector.tensor_add(bias_sb, bias_ps, braw)
                if qscale is not None:
                    nc.vector.tensor_scalar_mul(bias_sb, bias_sb, qscale)
                keep_warm(4)
                b_pm = []
                for ct in range(CT):
                    bp_ps = pspool.tile([P, 1], F32, name=f"{bnm}_pps",
                                        tag="bias")
                    nc.tensor.matmul(bp_ps,
                                     lhsT=bias_sb[0:1, ct * P:(ct + 1) * P],
                                     rhs=one11, start=True, stop=True)
                    bp = pmisc.tile([P, 1], F32, name=f"{bnm}4_{ct}")
                    nc.vector.tensor_copy(bp, bp_ps)
                    b_pm.append(bp)
                return wf, bias_sb, b_pm

            with tc.tile_pool(name="pw", bufs=1) as pw, \
                 tc.tile_pool(name="pwraw", bufs=1) as pwraw, \
                 tc.tile_pool(name="psc", bufs=2, space="PSUM") as psc:
                wk_f, _, bk4 = fold_weight("wk", a4, None, pw, psc, pwraw)
                wq_f, _, bq4 = fold_weight(
                    "wq", aq4, 1.0 / float(np.sqrt(C)), pw, psc, pwraw)
                wv_f, bv_sb, _ = fold_weight("wv", a4, None, pw, psc, pwraw)
                bv_b = pmisc.tile([P, C], F32, name="bv_b")
                nc.gpsimd.partition_broadcast(bv_b, bv_sb)

                # ==== Phase D: projections from resident x^T ====
                # K^T[co][:, chunk] = sum_ct wk'[ct][:,co*128:] ^T @ x^T[ct]
                for ch in range(NCH):
                    for co in range(CT):
                        kps = psp.tile([P, CHUNK], F32, name="kps", tag="kps")
                        for ct in range(CT):
                            nc.tensor.matmul(
                                kps, lhsT=wk_f[ct][:, co * P:(co + 1) * P],
                                rhs=xt[ct][:, ch * CHUNK:(ch + 1) * CHUNK],
                                start=(ct == 0), stop=(ct == CT - 1))
                        nc.scalar.activation(
                            kt[co][:, ch * CHUNK:(ch + 1) * CHUNK], kps,
                            AF.Identity, bias=bk4[co], scale=1.0)
                for ch in range(QCH):
                    for co in range(CT):
                        qps = psp.tile([P, CHUNK], F32, name="qps", tag="kps")
                        for ct in range(CT):
                            nc.tensor.matmul(
                                qps, lhsT=wq_f[ct][:, co * P:(co + 1) * P],
                                rhs=xtq[ct][:, ch * CHUNK:(ch + 1) * CHUNK],
                                start=(ct == 0), stop=(ct == CT - 1))
                        nc.scalar.activation(
                            qt[co][:, ch * CHUNK:(ch + 1) * CHUNK], qps,
                            AF.Identity, bias=bq4[co], scale=1.0)
                # V natural: lhsT = x^T pixel-block, rhs = wv'
                for nt_i in range(NT):
                    vps = psp.tile([P, C], F32, name="vps", tag="kps")
                    for ct in range(CT):
                        nc.tensor.matmul(
                            vps, lhsT=xt[ct][:, nt_i * P:(nt_i + 1) * P],
                            rhs=wv_f[ct], start=(ct == 0), stop=(ct == CT - 1))
                    nc.vector.tensor_add(vv[nt_i], vps, bv_b)

        # ==== Phase E/F: attention + output projection ====
        with tc.tile_pool(name="pwo", bufs=1) as pwo, \
             tc.tile_pool(name="pres", bufs=1) as pres, \
             tc.tile_pool(name="pe", bufs=3) as pe, \
             tc.tile_pool(name="pef", bufs=2) as pef, \
             tc.tile_pool(name="pss", bufs=2, space="PSUM") as pss, \
             tc.tile_pool(name="psat", bufs=1, space="PSUM") as psat, \
             tc.tile_pool(name="psr", bufs=1, space="PSUM") as psr, \
             tc.tile_pool(name="pso", bufs=1, space="PSUM") as pso:
            wo_f = []
            for ct in range(CT):
                raw = pef.tile([P, C], F32, name="wo_raw", tag="wo_raw")
                nc.sync.dma_start(out=raw, in_=w_d["wo"][ct * P:(ct + 1) * P, :])
                wo_t = pwo.tile([P, C], BF16, name=f"wo_f{ct}", tag=f"wo_f{ct}")
                nc.scalar.copy(wo_t, raw)
                wo_f.append(wo_t)
            bo_raw = pef.tile([1, C], F32, name="bo_raw", tag="bo_raw")
            nc.sync.dma_start(out=bo_raw, in_=b_d["bo"][:])
            bo_b = pwo.tile([P, C], F32, name="bo_b", tag="bo_b")
            nc.gpsimd.partition_broadcast(bo_b, bo_raw)
            resb = []
            for i in range(QTILES):
                rraw = pef.tile([P, C], F32, name="rraw", tag="rraw")
                nc.sync.dma_start(out=rraw, in_=x_res[i * P:(i + 1) * P, :])
                rb = pres.tile([P, C], F32, name=f"resb{i}", tag=f"resb{i}")
                nc.vector.tensor_add(rb, rraw, bo_b)
                resb.append(rb)

            at_ps = [psat.tile([P, CHUNK], F32, name=f"at{i}", tag=f"at{i}")
                     for i in range(CT)]
            for qc in range(QCH):
                rows_ps = psr.tile([1, CHUNK], F32, name="rows", tag="rows")
                for kt_i in range(NT):
                    sc_ps = pss.tile([P, CHUNK], F32, name="sc", tag="sc")
                    for ct in range(CT):
                        nc.tensor.matmul(
                            sc_ps,
                            lhsT=kt[ct][:, kt_i * P:(kt_i + 1) * P],
                            rhs=qt[ct][:, qc * CHUNK:(qc + 1) * CHUNK],
                            start=(ct == 0), stop=(ct == CT - 1))
                    probs = pe.tile([P, CHUNK], BF16, name="probs", tag="probs")
                    nc.scalar.activation(probs, sc_ps, AF.Exp)
                    for co in range(CT):
                        nc.tensor.matmul(
                            at_ps[co],
                            lhsT=vv[kt_i][:, co * P:(co + 1) * P],
                            rhs=probs,
                            start=(kt_i == 0), stop=(kt_i == NT - 1))
                    nc.tensor.matmul(rows_ps, lhsT=ones_bf, rhs=probs,
                                     start=(kt_i == 0), stop=(kt_i == NT - 1))
                # softmax denominators -> per-partition reciprocals
                recip = pe.tile([1, CHUNK], F32, name="recip", tag="recip")
                nc.vector.reciprocal(recip, rows_ps)
                recip4 = []
                for qi in range(4):
                    r4_ps = psr.tile([P, 1], F32, name="r4", tag="rows")
                    nc.tensor.matmul(r4_ps,
                                     lhsT=recip[0:1, qi * P:(qi + 1) * P],
                                     rhs=one11, start=True, stop=True)
                    r4 = pe.tile([P, 1], F32, name="recip4", tag=f"recip4_{qi}")
                    nc.vector.tensor_copy(r4, r4_ps)
                    recip4.append(r4)
                # unnormalized attn^T -> SBUF (no dependency on rowsums)
                at_sb = []
                for co in range(CT):
                    a_sb = pe.tile([P, CHUNK], BF16, name="at_sb",
                                   tag=f"at_sb{co}")
                    nc.scalar.copy(a_sb, at_ps[co])
                    at_sb.append(a_sb)
                for qi in range(4):
                    ops = pso.tile([P, C], F32, name="ops", tag="ops")
                    for ct in range(CT):
                        nc.tensor.matmul(
                            ops, lhsT=at_sb[ct][:, qi * P:(qi + 1) * P],
                            rhs=wo_f[ct], start=(ct == 0), stop=(ct == CT - 1))
                    fin = pe.tile([P, C], F32, name="fin", tag="fin")
                    # normalize rows here: out_row *= 1/rowsum (per-partition)
                    nc.scalar.activation(fin, ops, AF.Copy, bias=0.0,
                                         scale=recip4[qi])
                    fin2 = pe.tile([P, C], F32, name="fin2", tag="fin2")
                    nc.vector.tensor_add(fin2, fin, resb[qc * 4 + qi])
                    r0 = (qc * 4 + qi) * P
                    nc.sync.dma_start(out=out_d[r0:r0 + P, :], in_=fin2)

    nc.compile()
    return nc


def _consts():
    ident = np.eye(P, dtype=np.float32)
    gind = np.zeros((P, 8), dtype=np.float32)
    for p in range(P):
        gind[p, p // GS] = 1.0
    gindt = np.ascontiguousarray(gind.T)
    return ident, gind, gindt


def _make_in_maps(inputs):
    x = np.ascontiguousarray(np.asarray(inputs["inputs"], dtype=np.float32))
    xf = x.reshape(B, N, C)
    ident, gind, gindt = _consts()
    shared = {
        "gamma": np.ascontiguousarray(np.asarray(inputs["gn_gamma"], np.float32)),
        "beta": np.ascontiguousarray(np.asarray(inputs["gn_beta"], np.float32)),
        "ident": ident, "gind": gind, "gindt": gindt,
    }
    for nm in ("wq", "wk", "wv", "wo", "bq", "bk", "bv", "bo"):
        shared[nm] = np.ascontiguousarray(np.asarray(inputs[nm], np.float32))
    in_maps = []
    for core in range(NCORES):
        b, qq = divmod(core, 4)
        xr = np.ascontiguousarray(xf[b, qq * QS:(qq + 1) * QS, :])
        m = dict(shared)
        m["x_full"] = np.ascontiguousarray(xf[b])
        m["x_res"] = xr
        m["x_resr"] = xr
        in_maps.append(m)
    return in_maps


def _assemble(results):
    out = np.empty((B, N, C), dtype=np.float32)
    for core in range(NCORES):
        b, qq = divmod(core, 4)
        out[b, qq * QS:(qq + 1) * QS, :] = results[core]["out"]
    return out.reshape(B, HH, WW, C)


def kernel(**inputs):
    global _NC_CACHE
    if _NC_CACHE is None:
        _NC_CACHE = _build()
    in_maps = _make_in_maps(inputs)
    res = run_bass_kernel_spmd(_NC_CACHE, in_maps, list(range(NCORES)))
    return _assemble(res.results)


def _install_ntff_shim():
    """The agent image's antenv lacks axon_hooks; provide it so
    run_bass_kernel_spmd(trace=True) can NTFF-profile through axon."""
    import types
    import antenv
    if "antenv.axon_hooks" in sys.modules:
        return
    mod = types.ModuleType("antenv.axon_hooks")
    mod._hook = None

    def set_axon_ntff_profile_hook(h):
        mod._hook = h

    def get_axon_ntff_profile_hook():
        return mod._hook

    mod.set_axon_ntff_profile_hook = set_axon_ntff_profile_hook
    mod.get_axon_ntff_profile_hook = get_axon_ntff_profile_hook
    sys.modules["antenv.axon_hooks"] = mod
    antenv.axon_hooks = mod
    sys.path.insert(0, "/root/.axon_site")
    from trn_agent_boot.trn_boot import _ntff_profile_via_ctypes
    hook = _ntff_profile_via_ctypes("/opt/axon/libaxon_pjrt.so")
    set_axon_ntff_profile_hook(hook)


def run_traced(inputs, trace_kwargs=None):
    """Traced run for profiling: returns (BassKernelResults, tmpdir)."""
    global _NC_CACHE
    if _NC_CACHE is None:
        _NC_CACHE = _build()
    import tempfile
    _install_ntff_shim()
    in_maps = _make_in_maps(inputs)
    tmpdir = tempfile.mkdtemp(prefix="trace_")
    res = run_bass_kernel_spmd(_NC_CACHE, in_maps, list(range(NCORES)),
                               trace=True, tmpdir=tmpdir,
                               trace_kwargs=trace_kwargs or {})
    return res, tmpdir
